# revision 32
# baseline (speedup 1.0000x reference)
"""ExplaiNN (dense_cnn) Trainium2 Bass kernel, 8-core SPMD. v2.

Pipeline per reference:
  conv1d(4->300 units, K=19) + BN1 + exp + maxpool(7) -> per-unit fc1 (83->100)
  + BN2 + relu -> per-unit fc2 (100->1) + BN3 + relu -> final linear (300->2).

Distribution: conv+pool batch-sharded (16 b/core, all units), then an AllToAll
exchanges pooled features so fc1/fc2/final run unit-sharded (38 u/core, full
batch 128).  Final [128,2] partials are summed on host.

v2 vs v1:
  - conv in bf16 (fp32r was ~4x slower + no FWL on LDWEIGHTS)
  - BN1 affine folded into conv weights (a1*w) + a ones-row carrying c1,
    so psum is already normalized; exp applied post-pool with no scale/bias
  - maxpool split across engines: DVE reduce_max direct from PSUM for some
    batches, DVE/ACT copy to SBUF + GpSimd pairwise-max tree for the rest
  - pexp -> poolT transpose via pipelined PE transposes (identity matmul)
    + batched DVE evacuation, replacing 48 serial DMA_TRANSPOSEs (60us of
    HWDGE sequencer occupancy)
  - fc1 weights padded 100->128 h cols so LDWEIGHTS gets FWL
"""

import numpy as np
import ml_dtypes

B, N, L, K, C1 = 128, 300, 600, 19, 100
PS = 7
LP = 83            # pool windows
LC2 = 582          # psum conv cols (581 needed, +1 garbage for even splits)
NCLS = 2
EPS = 1e-5

NCORES = 8
BLOC = B // NCORES            # 16 batch per core in phase A
NPAD = 304                    # units padded to 8*38
ULOC = NPAD // NCORES         # 38 units per core in phase B
CK = 76                       # 4*19 contraction rows
WCONV_COLS = 384              # conv weight cols padded so every matmul is M=128
QP = 96                       # pexp q-cols padded (83 pools + ones col at 83)
HPAD = 128                    # fc1 h padded 100->128 (FWL needs 128 weight cols)

# packed-weight column offsets (full 304-unit fc weights, batch-sharded fc)
NR = NPAD // 4                # fc rounds: 4 col-tiled units per round
W_CONV = 0
W_W1 = 384
W_W2 = W_W1 + NPAD * 128      # 39296
W_FW = W_W2 + NPAD            # 39600
W_ID = W_FW + NCLS * NR       # 39752
WTOT = W_ID + 128             # 39880

# conv matmul column splits for batch-pairs: window-aligned 36/36/11 pool
# windows, each [128, 2, n] fp32 tile fits a single PSUM bank
CSPLIT = [(0, 252), (252, 252), (504, 78)]

# per-batch-PAIR pool mode (uniform within a pair): 'd'=DVE reduce_max
# direct, 'a'=ACT copy to SBUF (w-major) + DVE bf16 pairwise-max tree
POOL_MODES = "dddddddd" + "aaaaaaaa"

_CACHE = {}


def _build_bass():
    import concourse.bass as bass
    import concourse.bacc as bacc
    import concourse.mybir as mybir
    import concourse.tile as tile

    f32, bf16 = mybir.dt.float32, mybir.dt.bfloat16

    nc = bacc.Bacc("TRN2")
    xloc = nc.declare_dram_parameter("xloc", [4, BLOC, L], bf16, isOutput=False)
    # packed bf16 weights: [wconv 0:384 | w1aug 384:5248 | w2aug 5248:5286 |
    #                       fwrep 5286:5362 | ident 5362:5490]
    wpack = nc.declare_dram_parameter("wpack", [128, WTOT], bf16, isOutput=False)
    c1p = nc.declare_dram_parameter("c1p", [128, 3], mybir.dt.float32, isOutput=False)
    out_part = nc.declare_dram_parameter("out_part", [128, NCLS], f32, isOutput=True)

    n_copy = sum(1 for m in POOL_MODES if m != 'd')   # copy-path slots per chunk
    b_copy0 = POOL_MODES.index('a')

    with tile.TileContext(nc) as tc:
        with (
            tc.tile_pool(name="dram", bufs=1, space="DRAM") as dram_pool,
            tc.tile_pool(name="singles", bufs=1) as singles,
            tc.tile_pool(name="im2col", bufs=1) as im2col_pool,
            tc.tile_pool(name="praw", bufs=1) as praw_pool,
            tc.tile_pool(name="praws", bufs=1) as praws_pool,
            tc.tile_pool(name="gpst", bufs=1) as gpst_pool,
            tc.tile_pool(name="pexp", bufs=1) as pexp_pool,
            tc.tile_pool(name="scratch", bufs=1, space="PSUM") as scratch_pool,
        ):
            # im2col: [76, 16, 600] bf16; row (c*19+k), col (b, l) reads the
            # c-major flat x at 600b + l + k, one contiguous run per
            # partition (l+k <= 599 so no b-row crossing is ever read).
            # Issued first: conv blocks on it.
            im2all = im2col_pool.tile([CK, BLOC, L], bf16, name="im2all")
            QL = 4 * L
            for bq in range(0, BLOC, 4):
                nrun = QL - (K - 1)
                src = bass.AP(
                    tensor=xloc,
                    offset=bq * L,
                    ap=[[BLOC * L, 4], [1, K], [1, nrun]],
                )
                nc.sync.dma_start(
                    out=im2all[:, bq:bq + 4, :].rearrange(
                        "p b l -> p (b l)")[:, 0:nrun],
                    in_=src)

            wp_sb = singles.tile([128, WTOT], bf16)
            nc.sync.dma_start(out=wp_sb[:, 0:W_W1], in_=wpack[:, 0:W_W1])
            c1_sb = singles.tile([128, 3], f32)
            nc.scalar.dma_start(out=c1_sb, in_=c1p[:, :])
            wconv_sb = wp_sb[0:CK, W_CONV:W_CONV + WCONV_COLS]
            ident_sb = wp_sb[0:128, W_ID:W_ID + 128]

            def load_wpack_rest():
                # big fc-weight load, issued only after conv is underway so
                # it doesn't compete with im2col for DMA bandwidth
                nc.sync.dma_start(out=wp_sb[0:LP + 1, W_W1:W_W2],
                                  in_=wpack[0:LP + 1, W_W1:W_W2])
                nc.scalar.dma_start(out=wp_sb[0:C1 + 1, W_W2:W_W2 + NPAD],
                                    in_=wpack[0:C1 + 1, W_W2:W_W2 + NPAD])
                nc.sync.dma_start(out=wp_sb[:, W_FW:WTOT],
                                  in_=wpack[:, W_FW:WTOT])

            praw = []       # pooled, BN1-normalized, pre-exp [128, 16, 83]
            praw_s = []     # raw conv rows staged for the GpSimd tree
            pexp = []       # exp'd pooled features [128, 16, 96], col 83 = ones
            for ci in range(3):
                praw.append(praw_pool.tile([128, BLOC, LP], bf16, name=f"praw{ci}"))
                praw_s.append(praws_pool.tile([128, n_copy, PS * LP], bf16,
                                              name=f"praws", tag="praws"))
                p = pexp_pool.tile([128, BLOC, QP], bf16, name=f"pexp{ci}")
                nc.vector.memset(p[:, :, LP:QP], 1.0)
                pexp.append(p)

            def absorb(tile_ap):
                s = scratch_pool.tile([2, 2], f32, name="dummy", tag="dummy")
                src = tile_ap.bitcast(bf16) if tile_ap.dtype != bf16 else tile_ap
                src = src[0:1, 0:2]
                nc.tensor.matmul(out=s, lhsT=src, rhs=src,
                                 start=True, stop=True)

            # ---------------- conv + pool + transpose, chunk-pipelined ----
            # poolT[p, u, b] = pexp[u, b, p]; p-row 83 = ones (fc1 bias row)
            poolT = singles.tile([LP + 1, NPAD, BLOC], bf16)
            CH = (LP + 1) * ULOC * BLOC
            UB = ULOC * BLOC
            QSPLIT = [(0, 36), (36, 36), (72, 11)]   # window ranges

            def conv_chunk(ci):
                u0 = 128 * ci
                lhsT = wconv_sb[:, u0:u0 + 128]
                slot = 0
                for bp in range(0, BLOC, 2):
                    pss = [
                        pool_a.tile([128, 2, 252], f32, name="ps0", tag="ps0"),
                        pool_b.tile([128, 2, 252], f32, name="ps1", tag="ps1"),
                        pool_c.tile([128, 2, 78], f32, name="ps2", tag="ps2"),
                    ]
                    for pst, (l0, ncol) in zip(pss, CSPLIT):
                        nc.tensor.matmul(
                            out=pst[:, :, :],
                            lhsT=lhsT,
                            rhs=im2all[:, bp:bp + 2, l0:l0 + ncol],
                            start=True, stop=True,
                        )
                    mode = POOL_MODES[bp]
                    if mode == 'd':
                        for pst, (q0, nq) in zip(pss, QSPLIT):
                            nc.vector.reduce_max(
                                out=praw[ci][:, bp:bp + 2, q0:q0 + nq],
                                in_=pst[:, :, 0:nq * PS].rearrange(
                                    "p s (q w) -> p s q w", w=PS),
                                axis=mybir.AxisListType.X,
                            )
                    else:
                        # copy in w-major order so the DVE tree below is
                        # contiguous (bf16 2x): col w*83+q <- psum 7q+w
                        view = praw_s[ci][:, slot:slot + 2, :].rearrange(
                            "p s (w q) -> p s w q", q=LP)
                        for pst, (q0, nq) in zip(pss, QSPLIT):
                            nc.scalar.copy(
                                out=view[:, :, :, q0:q0 + nq],
                                in_=pst[:, :, 0:nq * PS].rearrange(
                                    "p s (q w) -> p s w q", w=PS),
                            )
                        slot += 2

                # DVE bf16 pairwise-max tree over the copy-path batches
                if n_copy:
                    s = praw_s[ci]
                    w_of = lambda w: s[:, :, w * LP:(w + 1) * LP]
                    tA = gpst_pool.tile([128, n_copy, LP], bf16, name="tA", tag="tA")
                    tB = gpst_pool.tile([128, n_copy, LP], bf16, name="tB", tag="tB")
                    tC = gpst_pool.tile([128, n_copy, LP], bf16, name="tC", tag="tC")
                    tD = gpst_pool.tile([128, n_copy, LP], bf16, name="tD", tag="tD")
                    tE = gpst_pool.tile([128, n_copy, LP], bf16, name="tE", tag="tE")
                    nc.vector.tensor_max(out=tA, in0=w_of(0), in1=w_of(1))
                    nc.vector.tensor_max(out=tB, in0=w_of(2), in1=w_of(3))
                    nc.vector.tensor_max(out=tC, in0=w_of(4), in1=w_of(5))
                    nc.vector.tensor_max(out=tD, in0=tA, in1=tB)
                    nc.vector.tensor_max(out=tE, in0=tC, in1=w_of(6))
                    nc.vector.tensor_max(
                        out=praw[ci][:, b_copy0:b_copy0 + n_copy, :],
                        in0=tD, in1=tE)

                # exp over the chunk's pooled features (normalized already)
                nc.scalar.activation(
                    out=pexp[ci][:, :, 0:LP],
                    in_=praw[ci][:, :, :],
                    func=mybir.ActivationFunctionType.Exp,
                    bias=c1_sb[:, ci:ci + 1],
                )

            def transpose_chunk(ci):
                u0 = 128 * ci
                un = min(128, NPAD - u0)       # 128,128,48
                for b0 in range(0, BLOC, 4):
                    pst = psumt_pool.tile([QP, 4, 128], bf16, name="psT", tag="psT")
                    for k in range(4):
                        nc.tensor.transpose(
                            out=pst[:, k, :],
                            in_=pexp[ci][:, b0 + k, :],
                            identity=ident_sb[:, :],
                        )
                    evac_out = poolT[0:LP + 1, u0:u0 + un, b0:b0 + 4]
                    evac_in = pst[0:LP + 1, :, 0:un].rearrange("p b u -> p u b")
                    if (b0 // 4) % 2 == 0:
                        nc.vector.tensor_copy(out=evac_out, in_=evac_in)
                    else:
                        nc.scalar.copy(out=evac_out, in_=evac_in)

            with (
                tc.tile_pool(name="psA0", bufs=2, space="PSUM") as pool_a,
                tc.tile_pool(name="psA1", bufs=2, space="PSUM") as pool_b,
                tc.tile_pool(name="psA2", bufs=1, space="PSUM") as pool_c,
                tc.tile_pool(name="psT", bufs=2, space="PSUM") as psumt_pool,
            ):
                absorb(wconv_sb[0:1, 0:2])
                for ci in range(3):
                    conv_chunk(ci)
                    if ci == 0:
                        load_wpack_rest()
                    if ci > 0:
                        transpose_chunk(ci - 1)
                transpose_chunk(2)

            # ---------------- fc1/fc2, batch-sharded ----------
            # fc1 orientation B: out[h, b] per unit; w1_u is the stationary
            # operand (FWL, 128 cols), pooled [84, 16] streams as rhs.
            # h2B[(h), (u, b)] feeds fc2 directly (h on partitions).
            w1_sb = wp_sb[0:LP + 1, W_W1:W_W1 + NPAD * HPAD]
            w2_sb = wp_sb[0:C1 + 1, W_W2:W_W2 + NPAD]
            fw_sb = wp_sb[0:128, W_FW:W_FW + NCLS * NR]

            h2B = singles.tile([128, NPAD * BLOC], bf16)
            nc.vector.memset(h2B[96:128, :], 1.0)   # row 100 = fc2 bias ones

            with (
                tc.tile_pool(name="psB", bufs=5, space="PSUM") as psum_b,
                tc.tile_pool(name="psF", bufs=1, space="PSUM") as psumf_pool,
            ):
                absorb(w1_sb[0:1, 0:2])
                for ub in range(0, NPAD, 8):
                    psf = psum_b.tile([HPAD, 8, BLOC], f32, name="psf", tag="psf")
                    for k in range(8):
                        u = ub + k
                        nc.tensor.matmul(
                            out=psf[:, k, :],
                            lhsT=w1_sb[:, u * HPAD:(u + 1) * HPAD],
                            rhs=poolT[:, u, :],
                            start=True, stop=True,
                        )
                    ev_out = h2B[0:C1, ub * BLOC:(ub + 8) * BLOC]
                    ev_in = psf[0:C1, :, :]
                    if (ub // 8) % 2 == 0:
                        nc.vector.tensor_scalar_max(out=ev_out, in0=ev_in,
                                                    scalar1=0.0)
                    else:
                        nc.scalar.activation(
                            out=ev_out, in_=ev_in,
                            func=mybir.ActivationFunctionType.Relu)

                # fc2: h3[(g,b), r] = relu(h2B[:, u-block].T @ w2_u),
                # 4 units concurrently in the 4 PE column groups
                psF = psumf_pool.tile([128, NR], f32, name="psF")
                for r in range(NR):
                    for g in range(4):
                        u = 4 * r + g
                        nc.tensor.matmul(
                            out=psF[32 * g:32 * g + BLOC, r:r + 1],
                            lhsT=h2B[0:C1 + 1, u * BLOC:(u + 1) * BLOC],
                            rhs=w2_sb[:, u:u + 1],
                            start=True, stop=True,
                            tile_position=(0, 32 * g),
                        )
                h3_sb = singles.tile([128, NR], bf16)
                nc.vector.tensor_scalar_max(out=h3_sb, in0=psF, scalar1=0.0)

            # ---------------- final linear (per-group partials; host sums
            # the 4 col-groups) ---------
            prod = singles.tile([128, NR], f32)
            osum = singles.tile([128, NCLS], f32)
            for cls in range(NCLS):
                nc.vector.tensor_mul(out=prod, in0=h3_sb,
                                     in1=fw_sb[:, cls * NR:(cls + 1) * NR])
                nc.vector.reduce_sum(
                    out=osum[:, cls:cls + 1], in_=prod,
                    axis=mybir.AxisListType.X,
                )
            nc.sync.dma_start(out=out_part[:, :], in_=osum)

    nc.finalize()
    return nc


def _host_prep(inputs):
    """Fold BN affines, pad units to 304, build per-core input maps."""
    x = np.asarray(inputs["x"], np.float32)
    conv_w = np.asarray(inputs["conv_w"], np.float32)
    conv_b = np.asarray(inputs["conv_b"], np.float32)
    g1, b1 = np.asarray(inputs["bn1_g"], np.float32), np.asarray(inputs["bn1_b"], np.float32)
    m1, v1 = np.asarray(inputs["bn1_m"], np.float32), np.asarray(inputs["bn1_v"], np.float32)
    fc1_w, fc1_b = np.asarray(inputs["fc1_w"], np.float32), np.asarray(inputs["fc1_b"], np.float32)
    g2, b2 = np.asarray(inputs["bn2_g"], np.float32), np.asarray(inputs["bn2_b"], np.float32)
    m2, v2 = np.asarray(inputs["bn2_m"], np.float32), np.asarray(inputs["bn2_v"], np.float32)
    fc2_w, fc2_b = np.asarray(inputs["fc2_w"], np.float32), np.asarray(inputs["fc2_b"], np.float32)
    g3, b3 = np.asarray(inputs["bn3_g"], np.float32), np.asarray(inputs["bn3_b"], np.float32)
    m3, v3 = np.asarray(inputs["bn3_m"], np.float32), np.asarray(inputs["bn3_v"], np.float32)
    final_w = np.asarray(inputs["final_w"], np.float32)
    final_b = np.asarray(inputs["final_b"], np.float32)

    a1 = g1 / np.sqrt(v1 + EPS)                      # [300] > 0
    c1 = a1 * (conv_b - m1) + b1                     # [300]
    a2 = g2 / np.sqrt(v2 + EPS)                      # [300,100]
    c2 = b2 - a2 * m2 + a2 * fc1_b                   # [300,100]
    a3 = g3 / np.sqrt(v3 + EPS)                      # [300]
    c3 = a3 * (fc2_b - m3) + b3                      # [300]

    bf = ml_dtypes.bfloat16

    # conv weights [76, 384]: a1 folded in; cols >= 300 are zero pad
    wconv = np.zeros((CK, WCONV_COLS), np.float32)
    wconv[0:76, :N] = (conv_w * a1[:, None, None]).transpose(1, 2, 0).reshape(76, N)
    c1t = np.zeros((128, 3), np.float32)
    for ci in range(3):
        u0 = 128 * ci
        seg = c1[u0:min(u0 + 128, N)]
        c1t[0:len(seg), ci] = seg

    # fc1: lhsT [84, 128] per unit; rows 0..82 = a2*w1 (p-major),
    # row 83 = c2 (pairs with the ones row of pTall); h cols 100..127 zero
    w1aug = np.zeros((NPAD, LP + 1, HPAD), np.float32)
    w1aug[:N, :LP, :C1] = (fc1_w * a2[:, :, None]).transpose(0, 2, 1)
    w1aug[:N, LP, :C1] = c2

    # fc2: rhs [101, 1] per unit; rows 0..99 = a3*w2, row 100 = c3
    w2aug = np.zeros((NPAD, C1 + 1), np.float32)
    w2aug[:N, :C1] = fc2_w * a3[:, None]
    w2aug[:N, C1] = c3

    fwpad = np.zeros((NCLS, NPAD), np.float32)
    fwpad[:, :N] = final_w

    identity = np.eye(128, dtype=np.float32)

    w1c = w1aug.transpose(1, 0, 2).reshape(LP + 1, NPAD * HPAD)
    w2c = w2aug.T                                       # [101, 304]
    # fw2[32g+b, cls*NR+r] = final_w[cls, 4r+g]
    fw2 = np.zeros((128, NCLS * NR), np.float32)
    for g in range(4):
        for r in range(NR):
            for cls in range(NCLS):
                fw2[32 * g:32 * g + BLOC, cls * NR + r] = fwpad[cls, 4 * r + g]
    wp = np.zeros((128, WTOT), np.float32)
    wp[0:CK, W_CONV:W_CONV + WCONV_COLS] = wconv
    wp[0:LP + 1, W_W1:W_W1 + NPAD * HPAD] = w1c
    wp[0:C1 + 1, W_W2:W_W2 + NPAD] = w2c
    wp[:, W_FW:W_FW + NCLS * NR] = fw2
    wp[:, W_ID:W_ID + 128] = identity
    wp_bf = wp.astype(bf)

    in_maps = []
    for i in range(NCORES):
        in_maps.append({
            "xloc": np.ascontiguousarray(x[i * BLOC:(i + 1) * BLOC].transpose(1, 0, 2)).astype(bf),
            "wpack": wp_bf,
            "c1p": c1t,
        })
    return in_maps, final_b


def kernel(**inputs):
    from concourse.bass_utils import run_bass_kernel_spmd

    if "nc" not in _CACHE:
        _CACHE["nc"] = _build_bass()
    nc = _CACHE["nc"]

    in_maps, final_b = _host_prep(inputs)
    res = run_bass_kernel_spmd(nc, in_maps, core_ids=list(range(NCORES)))
    out = np.zeros((B, NCLS), np.float32)
    for i, r in enumerate(res.results):
        o = r["out_part"].reshape(4, 32, NCLS)[:, 0:BLOC, :]  # [g, b, cls]
        out[i * BLOC:(i + 1) * BLOC] = o.sum(axis=0)
    out += final_b[None, :]
    return out
